# revision 1
# baseline (speedup 1.0000x reference)
"""MixtureOfSoftMaxACF Trainium2 kernel.

Per-core (data-parallel over BS=8 across 8 cores, batch b per core):
  qt[b] memory reinterpreted as QQ[2, 2048, 64] (contiguous halves), same kt.
  For m in {0,1}:  S_m = QQ[m] @ KK[m].T / sqrt(128);  P_m = softmax(S_m, axis=-1)
  out[b] = (p0 * P_0 + p1 * P_1) @ vt[b]
  p: mixture prior (softmax over batch axis) -> computed on host, passed per-core.

Device pipeline per core:
  - Stage qt/kt as [128, 16, (m,d)] so one PE transpose per key-chunk yields
    both mixtures' d-major columns partition-aligned with the QT/KT layout
    (rows 0-63 = mixture 0 d's, 64-127 = mixture 1); DVE-copy PSUM->SBUF.
  - Scores: S^T [128 keys, 1024 q] = lhsT(K^T chunk [64,128]) @ rhs(Q^T slab), fp32r.
  - exp on ScalarE straight from PSUM -> E in SBUF (fp32r), scale=1/sqrt(128).
  - AV (V-stationary): outT[128 dv, q] += V_c-stationary matmul, rhs=E, N=512.
  - Denominator: D_rep[128, q] += ones[128,128]-stationary @ E (each row = D).
  - Normalize in the [dv, q] domain (partition-aligned elementwise), combine
    mixtures with prior, PE-transpose back to [q, dv], DVE copy, DMA out.
"""

import math
from contextlib import ExitStack

import numpy as np

import concourse.bass as bass
import concourse.bacc as bacc
import concourse.mybir as mybir
import concourse.tile as tile
from concourse.bass_utils import run_bass_kernel_spmd
from concourse.masks import make_identity

BS = 8
N = 2048          # queries
NK = 2048         # keys
DK = 128
M = 2
D = DK // M       # 64
DV = 128
TEMP = math.sqrt(DK)
NCH = NK // 128   # 16 key chunks
QH = 2            # query halves
QHN = N // QH     # 1024

F32 = mybir.dt.float32
F32R = mybir.dt.float32r

_NC = None
LAST_RESULT = None  # BassKernelResults of last run (test.py reads this)


def _build():
    nc = bacc.Bacc(None)
    qt_d = nc.declare_dram_parameter("qt_b", [N, DK], F32, isOutput=False)
    kt_d = nc.declare_dram_parameter("kt_b", [NK, DK], F32, isOutput=False)
    vt_d = nc.declare_dram_parameter("vt_b", [NK, DK], F32, isOutput=False)
    pr_d = nc.declare_dram_parameter("pr_b", [1, M], F32, isOutput=False)
    out_d = nc.declare_dram_parameter("out_b", [N, DK], F32, isOutput=True)

    with ExitStack() as ctx:
        tc = ctx.enter_context(tile.TileContext(nc))
        const = ctx.enter_context(tc.tile_pool(name="const", bufs=1))
        sbig = ctx.enter_context(tc.tile_pool(name="sbig", bufs=1))
        epool = ctx.enter_context(tc.tile_pool(name="epool", bufs=3))
        npool = ctx.enter_context(tc.tile_pool(name="npool", bufs=2))
        ps_s = ctx.enter_context(tc.tile_pool(name="ps_s", bufs=2, space="PSUM"))
        ps_acc = ctx.enter_context(tc.tile_pool(name="ps_acc", bufs=1, space="PSUM"))
        ps_d = ctx.enter_context(tc.tile_pool(name="ps_d", bufs=1, space="PSUM"))

        # ---- constants ----
        ident_f = const.tile([128, 128], F32)
        make_identity(nc, ident_f)
        ones_f = const.tile([128, 128], F32)
        nc.vector.memset(ones_f, 1.0)
        ones_w = const.tile([128, 128], F32R)
        nc.vector.tensor_copy(ones_w, ones_f)
        pr_sb = const.tile([128, M], F32)
        nc.sync.dma_start(
            out=pr_sb,
            in_=bass.AP(tensor=pr_d, offset=0, ap=[[0, 128], [1, M]]),
        )

        # ---- input staging: [128, 16, (m,d)] so stage[:, c, :] is a [128, 128]
        # block whose transpose has mixture m's d-rows at partitions m*64..m*64+63.
        # stage[p, c, m*64+d] = flat[m*131072 + (c*128+p)*64 + d]
        stages = []
        for src in (qt_d, kt_d):
            t = sbig.tile([128, NCH, DK], F32, tag=f"stage{len(stages)}")
            for m in range(M):
                nc.sync.dma_start(
                    out=t[:, :, m * D:(m + 1) * D],
                    in_=bass.AP(
                        tensor=src, offset=m * N * D,
                        ap=[[D, 128], [128 * D, NCH], [1, D]],
                    ),
                )
            stages.append(t)

        # V: [128, 16, 128]  (p, c, dv) <- vt[c*128+p, dv]
        v_st = sbig.tile([128, NCH, DV], F32)
        nc.sync.dma_start(
            out=v_st,
            in_=bass.AP(tensor=vt_d, offset=0,
                        ap=[[DK, 128], [128 * DK, NCH], [1, DV]]),
        )
        v_sb = sbig.tile([128, NCH, DV], F32R)
        nc.vector.tensor_copy(v_sb, v_st)

        # ---- phase 1: QT/KT [128, 2048] (rows m*64+d), via PE transpose + DVE copy ----
        qt_t = sbig.tile([128, N], F32R)
        kt_t = sbig.tile([128, NK], F32R)
        for stage, dst in ((stages[0], qt_t), (stages[1], kt_t)):
            for c in range(NCH):
                tp = ps_s.tile([128, 128], F32, tag="s")
                nc.tensor.transpose(tp, stage[:, c, :], ident_f)
                nc.vector.tensor_copy(dst[:, c * 128:(c + 1) * 128], tp)

        # ---- phase 2+3: attention ----
        scale = 1.0 / TEMP
        for qh in range(QH):
            outTn = []
            for m in range(M):
                outT = ps_acc.tile([128, QHN], F32, tag="outT")
                Drep = ps_d.tile([128, QHN], F32, tag="D")
                for c in range(NCH):
                    s = ps_s.tile([128, QHN], F32, tag="s")
                    for hf in range(2):
                        sl = slice(hf * 512, (hf + 1) * 512)
                        nc.tensor.matmul(
                            s[:, sl],
                            lhsT=kt_t[m * D:(m + 1) * D, c * 128:(c + 1) * 128],
                            rhs=qt_t[m * D:(m + 1) * D,
                                     qh * QHN + hf * 512: qh * QHN + (hf + 1) * 512],
                            start=True, stop=True,
                        )
                    E = epool.tile([128, QHN], F32R, tag="E")
                    nc.scalar.activation(E, s, mybir.ActivationFunctionType.Exp,
                                         scale=scale)
                    for hf in range(2):
                        sl = slice(hf * 512, (hf + 1) * 512)
                        nc.tensor.matmul(outT[:, sl], lhsT=v_sb[:, c, :], rhs=E[:, sl],
                                         start=(c == 0), stop=(c == NCH - 1))
                        nc.tensor.matmul(Drep[:, sl], lhsT=ones_w, rhs=E[:, sl],
                                         start=(c == 0), stop=(c == NCH - 1))
                # normalize this mixture in the [dv, q] domain
                drec = npool.tile([128, QHN], F32, tag="drec")
                nc.vector.reciprocal(drec, Drep)
                otn = npool.tile([128, QHN], F32, tag=f"outTn{m}")
                nc.vector.tensor_mul(otn, outT, drec)
                outTn.append(otn)

            # combine mixtures with prior weights: rT2 = p0*outTn0 + p1*outTn1
            rT = npool.tile([128, QHN], F32, tag="rT")
            nc.vector.tensor_scalar_mul(rT, outTn[0], pr_sb[:, 0:1])
            rT2 = npool.tile([128, QHN], F32, tag="rT2")
            nc.vector.scalar_tensor_tensor(
                out=rT2, in0=outTn[1], scalar=pr_sb[:, 1:2], in1=rT,
                op0=mybir.AluOpType.mult, op1=mybir.AluOpType.add,
            )
            # transpose back to [q, dv], copy to SBUF, store
            res_ps = ps_s.tile([128, QHN], F32, tag="s")
            for t in range(QHN // 128):
                nc.tensor.transpose(res_ps[:, t * 128:(t + 1) * 128],
                                    rT2[:, t * 128:(t + 1) * 128], ident_f)
            res_sb = npool.tile([128, QHN], F32, tag="res")
            nc.vector.tensor_copy(res_sb, res_ps)
            nc.sync.dma_start(
                out=bass.AP(tensor=out_d, offset=qh * QHN * DK,
                            ap=[[DK, 128], [128 * DK, QHN // 128], [1, DV]]),
                in_=res_sb.rearrange("p (t d) -> p t d", d=DV),
            )
    return nc


def _get_nc():
    global _NC
    if _NC is None:
        _NC = _build()
        _NC.finalize()  # Bacc.compile(): event sems, reg alloc, wait legalization
    return _NC


def _prior(qt, kernel):
    bar_qt = qt.astype(np.float32).mean(axis=1)          # (BS, dk)
    logits = kernel.astype(np.float32) @ bar_qt.T        # (m, BS)
    z = logits - logits.max(axis=1, keepdims=True)
    ez = np.exp(z)
    pm = ez / ez.sum(axis=1, keepdims=True)              # softmax over batch axis
    return pm.reshape(-1)


def kernel(qt, kt, vt, kernel):
    global LAST_RESULT
    import os
    nc = _get_nc()
    prior_flat = _prior(qt, kernel)
    in_maps = []
    for b in range(BS):
        pr = np.array([[prior_flat[2 * b], prior_flat[2 * b + 1]]], dtype=np.float32)
        in_maps.append({
            "qt_b": np.ascontiguousarray(qt[b], dtype=np.float32),
            "kt_b": np.ascontiguousarray(kt[b], dtype=np.float32),
            "vt_b": np.ascontiguousarray(vt[b], dtype=np.float32),
            "pr_b": pr,
        })
    trace = bool(int(os.environ.get("KERNEL_TRACE", "0")))
    res = run_bass_kernel_spmd(nc, in_maps, list(range(BS)), trace=trace)
    LAST_RESULT = res
    out = np.stack([np.asarray(res.results[b]["out_b"]).reshape(N, DK) for b in range(BS)])
    return out.astype(np.float32)



# revision 7
# speedup vs baseline: 1.2525x; 1.2525x over previous
"""MixtureOfSoftMaxACF Trainium2 kernel.

Per-core (data-parallel over BS=8 across 8 cores, batch b per core):
  qt[b] memory reinterpreted as QQ[2, 2048, 64] (contiguous halves), same kt.
  For m in {0,1}:  S_m = QQ[m] @ KK[m].T / sqrt(128);  P_m = softmax(S_m, axis=-1)
  out[b] = (p0 * P_0 + p1 * P_1) @ vt[b]
  p: mixture prior (softmax over batch axis) -> computed on host, passed per-core.

Device pipeline per core:
  - Stage qt/kt as [128, 16, (m,d)] so one PE transpose per key-chunk yields
    both mixtures' d-major columns partition-aligned with the QT/KT layout
    (rows 0-63 = mixture 0 d's, 64-127 = mixture 1); DVE-copy PSUM->SBUF.
  - Scores: S^T [128 keys, 1024 q] = lhsT(K^T chunk [64,128]) @ rhs(Q^T slab), fp32r.
  - exp on ScalarE straight from PSUM -> E in SBUF (fp32r), scale=1/sqrt(128).
  - AV (V-stationary): outT[128 dv, q] += V_c-stationary matmul, rhs=E, N=512.
  - Denominator: D_rep[128, q] += ones[128,128]-stationary @ E (each row = D).
  - Normalize in the [dv, q] domain (partition-aligned elementwise), combine
    mixtures with prior, PE-transpose back to [q, dv], DVE copy, DMA out.
"""

import math
from contextlib import ExitStack

import numpy as np

import concourse.bass as bass
import concourse.bacc as bacc
import concourse.mybir as mybir
import concourse.tile as tile
from concourse.bass_utils import run_bass_kernel_spmd
from concourse.masks import make_identity

BS = 8
N = 2048          # queries
NK = 2048         # keys
DK = 128
M = 2
D = DK // M       # 64
DV = 128
TEMP = math.sqrt(DK)
NCH = NK // 128   # 16 key chunks
QH = 2            # query halves
QHN = N // QH     # 1024

F32 = mybir.dt.float32
F32R = mybir.dt.float32r
BF16 = mybir.dt.bfloat16

_NC = None
LAST_RESULT = None  # BassKernelResults of last run (test.py reads this)


def _build():
    nc = bacc.Bacc(None)
    qt_d = nc.declare_dram_parameter("qt_b", [N, DK], F32, isOutput=False)
    kt_d = nc.declare_dram_parameter("kt_b", [NK, DK], F32, isOutput=False)
    vt_d = nc.declare_dram_parameter("vt_b", [NK, DK], F32, isOutput=False)
    pr_d = nc.declare_dram_parameter("pr_b", [1, M], F32, isOutput=False)
    out_d = nc.declare_dram_parameter("out_b", [N, DK], F32, isOutput=True)

    with ExitStack() as ctx:
        tc = ctx.enter_context(tile.TileContext(nc))
        const = ctx.enter_context(tc.tile_pool(name="const", bufs=1))
        sbig = ctx.enter_context(tc.tile_pool(name="sbig", bufs=1))
        epool = ctx.enter_context(tc.tile_pool(name="epool", bufs=3))
        npool = ctx.enter_context(tc.tile_pool(name="npool", bufs=2))
        ps_s = ctx.enter_context(tc.tile_pool(name="ps_s", bufs=2, space="PSUM"))
        ps_acc = ctx.enter_context(tc.tile_pool(name="ps_acc", bufs=1, space="PSUM"))
        ps_d = ctx.enter_context(tc.tile_pool(name="ps_d", bufs=1, space="PSUM"))

        # ---- constants ----
        ident_f = const.tile([128, 128], F32)
        make_identity(nc, ident_f)
        ones_f = const.tile([128, 128], F32)
        nc.vector.memset(ones_f, 1.0)
        ones_w = const.tile([128, 128], BF16)
        nc.vector.tensor_copy(ones_w, ones_f)
        pr_sb = const.tile([128, M], F32)
        nc.sync.dma_start(
            out=pr_sb,
            in_=bass.AP(tensor=pr_d, offset=0, ap=[[0, 128], [1, M]]),
        )

        # ---- input staging: [128, 16, (m,d)] so stage[:, c, :] is a [128, 128]
        # block whose transpose has mixture m's d-rows at partitions m*64..m*64+63.
        # stage[p, c, m*64+d] = flat[m*131072 + (c*128+p)*64 + d]
        stages = []
        for src in (qt_d, kt_d):
            t = sbig.tile([128, NCH, DK], F32, tag=f"stage{len(stages)}")
            for m in range(M):
                nc.sync.dma_start(
                    out=t[:, :, m * D:(m + 1) * D],
                    in_=bass.AP(
                        tensor=src, offset=m * N * D,
                        ap=[[D, 128], [128 * D, NCH], [1, D]],
                    ),
                )
            stages.append(t)

        # V: [128, 16, 128]  (p, c, dv) <- vt[c*128+p, dv]
        v_st = sbig.tile([128, NCH, DV], F32)
        nc.sync.dma_start(
            out=v_st,
            in_=bass.AP(tensor=vt_d, offset=0,
                        ap=[[DK, 128], [128 * DK, NCH], [1, DV]]),
        )
        v_sb = sbig.tile([128, NCH, DV], BF16)
        nc.vector.tensor_copy(v_sb, v_st)

        # ---- phase 1: QT/KT [128, 2048] (rows m*64+d), via PE transpose + DVE copy ----
        qt_t = sbig.tile([128, N], BF16)
        kt_t = sbig.tile([128, NK], BF16)
        for stage, dst in ((stages[0], qt_t), (stages[1], kt_t)):
            for c in range(NCH):
                tp = ps_s.tile([128, 128], F32, tag="s")
                nc.tensor.transpose(tp, stage[:, c, :], ident_f)
                nc.vector.tensor_copy(dst[:, c * 128:(c + 1) * 128], tp)

        # ---- phase 2+3: attention ----
        scale = 1.0 / TEMP
        for qh in range(QH):
            outTn = []
            for m in range(M):
                outT = ps_acc.tile([128, QHN], F32, tag="outT")
                Drep = ps_d.tile([128, QHN], F32, tag="D")
                for c in range(NCH):
                    s = ps_s.tile([128, QHN], F32, tag="s")
                    for hf in range(2):
                        sl = slice(hf * 512, (hf + 1) * 512)
                        nc.tensor.matmul(
                            s[:, sl],
                            lhsT=kt_t[m * D:(m + 1) * D, c * 128:(c + 1) * 128],
                            rhs=qt_t[m * D:(m + 1) * D,
                                     qh * QHN + hf * 512: qh * QHN + (hf + 1) * 512],
                            start=True, stop=True,
                        )
                    E = epool.tile([128, QHN], BF16, tag="E")
                    nc.scalar.activation(E, s, mybir.ActivationFunctionType.Exp,
                                         scale=scale)
                    for hf in range(2):
                        sl = slice(hf * 512, (hf + 1) * 512)
                        nc.tensor.matmul(outT[:, sl], lhsT=v_sb[:, c, :], rhs=E[:, sl],
                                         start=(c == 0), stop=(c == NCH - 1))
                        nc.tensor.matmul(Drep[:, sl], lhsT=ones_w, rhs=E[:, sl],
                                         start=(c == 0), stop=(c == NCH - 1))
                # normalize this mixture in the [dv, q] domain
                drec = npool.tile([128, QHN], F32, tag="drec")
                nc.vector.reciprocal_approx_fast(drec, Drep)
                otn = npool.tile([128, QHN], F32, tag=f"outTn{m}")
                nc.vector.tensor_mul(otn, outT, drec)
                outTn.append(otn)

            # combine mixtures with prior weights: rT2 = p0*outTn0 + p1*outTn1
            rT = npool.tile([128, QHN], F32, tag="rT")
            nc.vector.tensor_scalar_mul(rT, outTn[0], pr_sb[:, 0:1])
            rT2 = npool.tile([128, QHN], F32, tag="rT2")
            nc.vector.scalar_tensor_tensor(
                out=rT2, in0=outTn[1], scalar=pr_sb[:, 1:2], in1=rT,
                op0=mybir.AluOpType.mult, op1=mybir.AluOpType.add,
            )
            # transpose back to [q, dv], copy to SBUF, store
            res_ps = ps_s.tile([128, QHN], F32, tag="s")
            for t in range(QHN // 128):
                nc.tensor.transpose(res_ps[:, t * 128:(t + 1) * 128],
                                    rT2[:, t * 128:(t + 1) * 128], ident_f)
            res_sb = npool.tile([128, QHN], F32, tag="res")
            nc.vector.tensor_copy(res_sb, res_ps)
            nc.sync.dma_start(
                out=bass.AP(tensor=out_d, offset=qh * QHN * DK,
                            ap=[[DK, 128], [128 * DK, QHN // 128], [1, DV]]),
                in_=res_sb.rearrange("p (t d) -> p t d", d=DV),
            )
    return nc


def _get_nc():
    global _NC
    if _NC is None:
        _NC = _build()
        _NC.finalize()  # Bacc.compile(): event sems, reg alloc, wait legalization
    return _NC


def _prior(qt, kernel):
    bar_qt = qt.astype(np.float32).mean(axis=1)          # (BS, dk)
    logits = kernel.astype(np.float32) @ bar_qt.T        # (m, BS)
    z = logits - logits.max(axis=1, keepdims=True)
    ez = np.exp(z)
    pm = ez / ez.sum(axis=1, keepdims=True)              # softmax over batch axis
    return pm.reshape(-1)


def kernel(qt, kt, vt, kernel):
    global LAST_RESULT
    import os
    nc = _get_nc()
    prior_flat = _prior(qt, kernel)
    in_maps = []
    for b in range(BS):
        pr = np.array([[prior_flat[2 * b], prior_flat[2 * b + 1]]], dtype=np.float32)
        in_maps.append({
            "qt_b": np.ascontiguousarray(qt[b], dtype=np.float32),
            "kt_b": np.ascontiguousarray(kt[b], dtype=np.float32),
            "vt_b": np.ascontiguousarray(vt[b], dtype=np.float32),
            "pr_b": pr,
        })
    trace = bool(int(os.environ.get("KERNEL_TRACE", "0")))
    res = run_bass_kernel_spmd(nc, in_maps, list(range(BS)), trace=trace)
    LAST_RESULT = res
    out = np.stack([np.asarray(res.results[b]["out_b"]).reshape(N, DK) for b in range(BS)])
    return out.astype(np.float32)



# revision 9
# speedup vs baseline: 1.2718x; 1.0155x over previous
"""MixtureOfSoftMaxACF Trainium2 kernel.

Per-core (data-parallel over BS=8 across 8 cores, batch b per core):
  qt[b] memory reinterpreted as QQ[2, 2048, 64] (contiguous halves), same kt.
  For m in {0,1}:  S_m = QQ[m] @ KK[m].T / sqrt(128);  P_m = softmax(S_m, axis=-1)
  out[b] = (p0 * P_0 + p1 * P_1) @ vt[b]
  p: mixture prior (softmax over batch axis) -> computed on host, passed per-core.

Device pipeline per core:
  - Stage qt/kt as [128, 16, (m,d)] so one PE transpose per key-chunk yields
    both mixtures' d-major columns partition-aligned with the QT/KT layout
    (rows 0-63 = mixture 0 d's, 64-127 = mixture 1); DVE-copy PSUM->SBUF.
  - Scores: S^T [128 keys, 1024 q] = lhsT(K^T chunk [64,128]) @ rhs(Q^T slab), fp32r.
  - exp on ScalarE straight from PSUM -> E in SBUF (fp32r), scale=1/sqrt(128).
  - AV (V-stationary): outT[128 dv, q] += V_c-stationary matmul, rhs=E, N=512.
  - Denominator: D_rep[128, q] += ones[128,128]-stationary @ E (each row = D).
  - Normalize in the [dv, q] domain (partition-aligned elementwise), combine
    mixtures with prior, PE-transpose back to [q, dv], DVE copy, DMA out.
"""

import math
from contextlib import ExitStack

import numpy as np

import concourse.bass as bass
import concourse.bacc as bacc
import concourse.mybir as mybir
import concourse.tile as tile
from concourse.bass_utils import run_bass_kernel_spmd
from concourse.masks import make_identity

BS = 8
N = 2048          # queries
NK = 2048         # keys
DK = 128
M = 2
D = DK // M       # 64
DV = 128
TEMP = math.sqrt(DK)
NCH = NK // 128   # 16 key chunks
QH = 2            # query halves
QHN = N // QH     # 1024

F32 = mybir.dt.float32
F32R = mybir.dt.float32r
BF16 = mybir.dt.bfloat16

_NC = None
LAST_RESULT = None  # BassKernelResults of last run (test.py reads this)


def _build():
    nc = bacc.Bacc(None)
    qt_d = nc.declare_dram_parameter("qt_b", [N, DK], F32, isOutput=False)
    kt_d = nc.declare_dram_parameter("kt_b", [NK, DK], F32, isOutput=False)
    vt_d = nc.declare_dram_parameter("vt_b", [NK, DK], F32, isOutput=False)
    pr_d = nc.declare_dram_parameter("pr_b", [1, M], F32, isOutput=False)
    out_d = nc.declare_dram_parameter("out_b", [N, DK], F32, isOutput=True)

    with ExitStack() as ctx:
        tc = ctx.enter_context(tile.TileContext(nc))
        const = ctx.enter_context(tc.tile_pool(name="const", bufs=1))
        sbig = ctx.enter_context(tc.tile_pool(name="sbig", bufs=1))
        epool = ctx.enter_context(tc.tile_pool(name="epool", bufs=3))
        npool = ctx.enter_context(tc.tile_pool(name="npool", bufs=2))
        ps_s = ctx.enter_context(tc.tile_pool(name="ps_s", bufs=2, space="PSUM"))
        ps_acc = ctx.enter_context(tc.tile_pool(name="ps_acc", bufs=1, space="PSUM"))
        ps_d = ctx.enter_context(tc.tile_pool(name="ps_d", bufs=1, space="PSUM"))

        # ---- constants ----
        ident_f = const.tile([128, 128], F32)
        make_identity(nc, ident_f)
        ones_f = const.tile([128, 128], F32)
        nc.vector.memset(ones_f, 1.0)
        ones_w = const.tile([128, 128], BF16)
        nc.vector.tensor_copy(ones_w, ones_f)
        pr_sb = const.tile([128, M], F32)
        nc.sync.dma_start(
            out=pr_sb,
            in_=bass.AP(tensor=pr_d, offset=0, ap=[[0, 128], [1, M]]),
        )

        # ---- input staging: [128, 16, (m,d)] so stage[:, c, :] is a [128, 128]
        # block whose transpose has mixture m's d-rows at partitions m*64..m*64+63.
        # stage[p, c, m*64+d] = flat[m*131072 + (c*128+p)*64 + d]
        stages = []
        for src in (qt_d, kt_d):
            t = sbig.tile([128, NCH, DK], F32, tag=f"stage{len(stages)}")
            for m in range(M):
                for h in range(4):
                    hc = NCH // 4
                    nc.sync.dma_start(
                        out=t[:, h * hc:(h + 1) * hc, m * D:(m + 1) * D],
                        in_=bass.AP(
                            tensor=src, offset=m * N * D + h * hc * 128 * D,
                            ap=[[D, 128], [128 * D, hc], [1, D]],
                        ),
                    )
            stages.append(t)

        # V: [128, 16, 128]  (p, c, dv) <- vt[c*128+p, dv]
        v_st = sbig.tile([128, NCH, DV], F32)
        nc.sync.dma_start(
            out=v_st,
            in_=bass.AP(tensor=vt_d, offset=0,
                        ap=[[DK, 128], [128 * DK, NCH], [1, DV]]),
        )
        v_sb = sbig.tile([128, NCH, DV], BF16)
        nc.vector.tensor_copy(v_sb, v_st)

        # ---- phase 1: QT/KT [128, 2048] (rows m*64+d), via PE transpose + DVE copy ----
        qt_t = sbig.tile([128, N], BF16)
        kt_t = sbig.tile([128, NK], BF16)
        for stage, dst in ((stages[0], qt_t), (stages[1], kt_t)):
            for c in range(NCH):
                tp = ps_s.tile([128, 128], F32, tag="s")
                nc.tensor.transpose(tp, stage[:, c, :], ident_f)
                nc.vector.tensor_copy(dst[:, c * 128:(c + 1) * 128], tp)

        # ---- phase 2+3: attention ----
        scale = 1.0 / TEMP
        for qh in range(QH):
            outTn = []
            for m in range(M):
                outT = ps_acc.tile([128, QHN], F32, tag="outT")
                Drep = ps_d.tile([128, QHN], F32, tag="D")
                for c in range(NCH):
                    s = ps_s.tile([128, QHN], F32, tag="s")
                    for hf in range(2):
                        sl = slice(hf * 512, (hf + 1) * 512)
                        nc.tensor.matmul(
                            s[:, sl],
                            lhsT=kt_t[m * D:(m + 1) * D, c * 128:(c + 1) * 128],
                            rhs=qt_t[m * D:(m + 1) * D,
                                     qh * QHN + hf * 512: qh * QHN + (hf + 1) * 512],
                            start=True, stop=True,
                        )
                    E = epool.tile([128, QHN], BF16, tag="E")
                    nc.scalar.activation(E, s, mybir.ActivationFunctionType.Exp,
                                         scale=scale)
                    for hf in range(2):
                        sl = slice(hf * 512, (hf + 1) * 512)
                        nc.tensor.matmul(outT[:, sl], lhsT=v_sb[:, c, :], rhs=E[:, sl],
                                         start=(c == 0), stop=(c == NCH - 1))
                        nc.tensor.matmul(Drep[:, sl], lhsT=ones_w, rhs=E[:, sl],
                                         start=(c == 0), stop=(c == NCH - 1))
                # normalize this mixture in the [dv, q] domain
                drec = npool.tile([128, QHN], F32, tag="drec")
                nc.vector.reciprocal_approx_fast(drec, Drep)
                otn = npool.tile([128, QHN], F32, tag=f"outTn{m}")
                nc.vector.tensor_mul(otn, outT, drec)
                outTn.append(otn)

            # combine mixtures with prior weights: rT2 = p0*outTn0 + p1*outTn1
            rT = npool.tile([128, QHN], F32, tag="rT")
            nc.vector.tensor_scalar_mul(rT, outTn[0], pr_sb[:, 0:1])
            rT2 = npool.tile([128, QHN], F32, tag="rT2")
            nc.vector.scalar_tensor_tensor(
                out=rT2, in0=outTn[1], scalar=pr_sb[:, 1:2], in1=rT,
                op0=mybir.AluOpType.mult, op1=mybir.AluOpType.add,
            )
            # transpose back to [q, dv], copy to SBUF, store — streamed per
            # 256-q piece so the final store DMA overlaps the transposes.
            res_ps = ps_s.tile([128, QHN], F32, tag="s")
            res_sb = npool.tile([128, QHN], F32, tag="res")
            res_v = res_sb.rearrange("p (t d) -> p t d", d=DV)
            for g in range(QHN // 256):
                for t in (2 * g, 2 * g + 1):
                    nc.tensor.transpose(res_ps[:, t * 128:(t + 1) * 128],
                                        rT2[:, t * 128:(t + 1) * 128], ident_f)
                sl = slice(g * 256, (g + 1) * 256)
                nc.vector.tensor_copy(res_sb[:, sl], res_ps[:, sl])
                nc.sync.dma_start(
                    out=bass.AP(tensor=out_d,
                                offset=(qh * QHN + g * 256) * DK,
                                ap=[[DK, 128], [128 * DK, 2], [1, DV]]),
                    in_=res_v[:, 2 * g:2 * g + 2, :],
                )
    return nc


def _get_nc():
    global _NC
    if _NC is None:
        _NC = _build()
        _NC.finalize()  # Bacc.compile(): event sems, reg alloc, wait legalization
    return _NC


def _prior(qt, kernel):
    bar_qt = qt.astype(np.float32).mean(axis=1)          # (BS, dk)
    logits = kernel.astype(np.float32) @ bar_qt.T        # (m, BS)
    z = logits - logits.max(axis=1, keepdims=True)
    ez = np.exp(z)
    pm = ez / ez.sum(axis=1, keepdims=True)              # softmax over batch axis
    return pm.reshape(-1)


def kernel(qt, kt, vt, kernel):
    global LAST_RESULT
    import os
    nc = _get_nc()
    prior_flat = _prior(qt, kernel)
    in_maps = []
    for b in range(BS):
        pr = np.array([[prior_flat[2 * b], prior_flat[2 * b + 1]]], dtype=np.float32)
        in_maps.append({
            "qt_b": np.ascontiguousarray(qt[b], dtype=np.float32),
            "kt_b": np.ascontiguousarray(kt[b], dtype=np.float32),
            "vt_b": np.ascontiguousarray(vt[b], dtype=np.float32),
            "pr_b": pr,
        })
    trace = bool(int(os.environ.get("KERNEL_TRACE", "0")))
    res = run_bass_kernel_spmd(nc, in_maps, list(range(BS)), trace=trace)
    LAST_RESULT = res
    out = np.stack([np.asarray(res.results[b]["out_b"]).reshape(N, DK) for b in range(BS)])
    return out.astype(np.float32)



# revision 10
# speedup vs baseline: 1.4538x; 1.1430x over previous
"""MixtureOfSoftMaxACF Trainium2 kernel.

Per-core (data-parallel over BS=8 across 8 cores, batch b per core):
  qt[b] memory reinterpreted as QQ[2, 2048, 64] (contiguous halves), same kt.
  For m in {0,1}:  S_m = QQ[m] @ KK[m].T / sqrt(128);  P_m = softmax(S_m, axis=-1)
  out[b] = (p0 * P_0 + p1 * P_1) @ vt[b]
  p: mixture prior (softmax over batch axis) -> computed on host, passed per-core.

Device pipeline per core:
  - Stage qt/kt as [128, 16, (m,d)] so one PE transpose per key-chunk yields
    both mixtures' d-major columns partition-aligned with the QT/KT layout
    (rows 0-63 = mixture 0 d's, 64-127 = mixture 1); DVE-copy PSUM->SBUF.
  - Scores: S^T [128 keys, 1024 q] = lhsT(K^T chunk [64,128]) @ rhs(Q^T slab), fp32r.
  - exp on ScalarE straight from PSUM -> E in SBUF (fp32r), scale=1/sqrt(128).
  - AV (V-stationary): outT[128 dv, q] += V_c-stationary matmul, rhs=E, N=512.
  - Denominator: D_rep[128, q] += ones[128,128]-stationary @ E (each row = D).
  - Normalize in the [dv, q] domain (partition-aligned elementwise), combine
    mixtures with prior, PE-transpose back to [q, dv], DVE copy, DMA out.
"""

import math
from contextlib import ExitStack

import numpy as np

import concourse.bass as bass
import concourse.bacc as bacc
import concourse.mybir as mybir
import concourse.tile as tile
from concourse.bass_utils import run_bass_kernel_spmd
from concourse.masks import make_identity

BS = 8
N = 2048          # queries
NK = 2048         # keys
DK = 128
M = 2
D = DK // M       # 64
DV = 128
TEMP = math.sqrt(DK)
NCH = NK // 128   # 16 key chunks
QH = 2            # query halves
QHN = N // QH     # 1024

F32 = mybir.dt.float32
F32R = mybir.dt.float32r
BF16 = mybir.dt.bfloat16

_NC = None
LAST_RESULT = None  # BassKernelResults of last run (test.py reads this)


def _build():
    nc = bacc.Bacc(None)
    qt_d = nc.declare_dram_parameter("qt_b", [N, DK], F32, isOutput=False)
    kt_d = nc.declare_dram_parameter("kt_b", [NK, DK], F32, isOutput=False)
    vt_d = nc.declare_dram_parameter("vt_b", [NK, DK], F32, isOutput=False)
    pr_d = nc.declare_dram_parameter("pr_b", [1, M], F32, isOutput=False)
    out_d = nc.declare_dram_parameter("out_b", [N, DK], F32, isOutput=True)

    with ExitStack() as ctx:
        tc = ctx.enter_context(tile.TileContext(nc))
        const = ctx.enter_context(tc.tile_pool(name="const", bufs=1))
        sbig = ctx.enter_context(tc.tile_pool(name="sbig", bufs=1))
        epool = ctx.enter_context(tc.tile_pool(name="epool", bufs=3))
        npool = ctx.enter_context(tc.tile_pool(name="npool", bufs=2))
        ps_s = ctx.enter_context(tc.tile_pool(name="ps_s", bufs=2, space="PSUM"))
        ps_acc = ctx.enter_context(tc.tile_pool(name="ps_acc", bufs=1, space="PSUM"))
        ps_d = ctx.enter_context(tc.tile_pool(name="ps_d", bufs=1, space="PSUM"))

        # ---- constants ----
        ident_f = const.tile([128, 128], F32)
        make_identity(nc, ident_f)
        ones_f = const.tile([128, 128], F32)
        nc.vector.memset(ones_f, 1.0)
        ones_w = const.tile([128, 128], BF16)
        nc.vector.tensor_copy(ones_w, ones_f)
        pr_sb = const.tile([128, M], F32)
        nc.sync.dma_start(
            out=pr_sb,
            in_=bass.AP(tensor=pr_d, offset=0, ap=[[0, 128], [1, M]]),
        )

        # ---- input staging: [128, 16, (m,d)] so stage[:, c, :] is a [128, 128]
        # block whose transpose has mixture m's d-rows at partitions m*64..m*64+63.
        # stage[p, c, m*64+d] = flat[m*131072 + (c*128+p)*64 + d]
        stages = []
        for src in (qt_d, kt_d):
            t = sbig.tile([128, NCH, DK], F32, tag=f"stage{len(stages)}")
            for m in range(M):
                for h in range(4):
                    hc = NCH // 4
                    nc.sync.dma_start(
                        out=t[:, h * hc:(h + 1) * hc, m * D:(m + 1) * D],
                        in_=bass.AP(
                            tensor=src, offset=m * N * D + h * hc * 128 * D,
                            ap=[[D, 128], [128 * D, hc], [1, D]],
                        ),
                    )
            stages.append(t)

        # V: [128, 16, 128]  (p, c, dv) <- vt[c*128+p, dv]
        v_st = sbig.tile([128, NCH, DV], F32)
        nc.sync.dma_start(
            out=v_st,
            in_=bass.AP(tensor=vt_d, offset=0,
                        ap=[[DK, 128], [128 * DK, NCH], [1, DV]]),
        )
        v_sb = sbig.tile([128, NCH, DV], BF16)
        nc.vector.tensor_copy(v_sb, v_st)

        # ---- phase 1: QT/KT [128, 2048] (rows m*64+d), via PE transpose + DVE copy ----
        qt_t = sbig.tile([128, N], BF16)
        kt_t = sbig.tile([128, NK], BF16)
        for stage, dst in ((stages[0], qt_t), (stages[1], kt_t)):
            for c in range(NCH):
                tp = ps_s.tile([128, 128], F32, tag="s")
                nc.tensor.transpose(tp, stage[:, c, :], ident_f)
                nc.vector.tensor_copy(dst[:, c * 128:(c + 1) * 128], tp)

        # ---- phase 2+3: attention ----
        scale = 1.0 / TEMP
        for qh in range(QH):
            outTn = []
            for m in range(M):
                outT = ps_acc.tile([128, QHN], F32, tag="outT")
                Drep = ps_d.tile([128, QHN], F32, tag="D")
                def emit_scores(c):
                    s = ps_s.tile([128, QHN], F32, tag="s")
                    for hf in range(2):
                        sl = slice(hf * 512, (hf + 1) * 512)
                        nc.tensor.matmul(
                            s[:, sl],
                            lhsT=kt_t[m * D:(m + 1) * D, c * 128:(c + 1) * 128],
                            rhs=qt_t[m * D:(m + 1) * D,
                                     qh * QHN + hf * 512: qh * QHN + (hf + 1) * 512],
                            start=True, stop=True,
                        )
                    return s

                # software pipeline: emit scores(c+1) on PE *before* AV(c)/
                # Drep(c) so the PE queue has work while ACT runs exp(c).
                s_cur = emit_scores(0)
                for c in range(NCH):
                    s_next = emit_scores(c + 1) if c + 1 < NCH else None
                    E = epool.tile([128, QHN], BF16, tag="E")
                    nc.scalar.activation(E, s_cur, mybir.ActivationFunctionType.Exp,
                                         scale=scale)
                    for hf in range(2):
                        sl = slice(hf * 512, (hf + 1) * 512)
                        nc.tensor.matmul(outT[:, sl], lhsT=v_sb[:, c, :], rhs=E[:, sl],
                                         start=(c == 0), stop=(c == NCH - 1))
                        nc.tensor.matmul(Drep[:, sl], lhsT=ones_w, rhs=E[:, sl],
                                         start=(c == 0), stop=(c == NCH - 1))
                    s_cur = s_next
                # normalize this mixture in the [dv, q] domain
                drec = npool.tile([128, QHN], F32, tag="drec")
                nc.vector.reciprocal_approx_fast(drec, Drep)
                otn = npool.tile([128, QHN], F32, tag=f"outTn{m}")
                nc.vector.tensor_mul(otn, outT, drec)
                outTn.append(otn)

            # combine mixtures with prior weights: rT2 = p0*outTn0 + p1*outTn1
            rT = npool.tile([128, QHN], F32, tag="rT")
            nc.vector.tensor_scalar_mul(rT, outTn[0], pr_sb[:, 0:1])
            rT2 = npool.tile([128, QHN], F32, tag="rT2")
            nc.vector.scalar_tensor_tensor(
                out=rT2, in0=outTn[1], scalar=pr_sb[:, 1:2], in1=rT,
                op0=mybir.AluOpType.mult, op1=mybir.AluOpType.add,
            )
            # transpose back to [q, dv], copy to SBUF, store — streamed per
            # 256-q piece so the final store DMA overlaps the transposes.
            res_ps = ps_s.tile([128, QHN], F32, tag="s")
            res_sb = npool.tile([128, QHN], F32, tag="res")
            res_v = res_sb.rearrange("p (t d) -> p t d", d=DV)
            for g in range(QHN // 256):
                for t in (2 * g, 2 * g + 1):
                    nc.tensor.transpose(res_ps[:, t * 128:(t + 1) * 128],
                                        rT2[:, t * 128:(t + 1) * 128], ident_f)
                sl = slice(g * 256, (g + 1) * 256)
                nc.vector.tensor_copy(res_sb[:, sl], res_ps[:, sl])
                nc.sync.dma_start(
                    out=bass.AP(tensor=out_d,
                                offset=(qh * QHN + g * 256) * DK,
                                ap=[[DK, 128], [128 * DK, 2], [1, DV]]),
                    in_=res_v[:, 2 * g:2 * g + 2, :],
                )
    return nc


def _get_nc():
    global _NC
    if _NC is None:
        _NC = _build()
        _NC.finalize()  # Bacc.compile(): event sems, reg alloc, wait legalization
    return _NC


def _prior(qt, kernel):
    bar_qt = qt.astype(np.float32).mean(axis=1)          # (BS, dk)
    logits = kernel.astype(np.float32) @ bar_qt.T        # (m, BS)
    z = logits - logits.max(axis=1, keepdims=True)
    ez = np.exp(z)
    pm = ez / ez.sum(axis=1, keepdims=True)              # softmax over batch axis
    return pm.reshape(-1)


def kernel(qt, kt, vt, kernel):
    global LAST_RESULT
    import os
    nc = _get_nc()
    prior_flat = _prior(qt, kernel)
    in_maps = []
    for b in range(BS):
        pr = np.array([[prior_flat[2 * b], prior_flat[2 * b + 1]]], dtype=np.float32)
        in_maps.append({
            "qt_b": np.ascontiguousarray(qt[b], dtype=np.float32),
            "kt_b": np.ascontiguousarray(kt[b], dtype=np.float32),
            "vt_b": np.ascontiguousarray(vt[b], dtype=np.float32),
            "pr_b": pr,
        })
    trace = bool(int(os.environ.get("KERNEL_TRACE", "0")))
    res = run_bass_kernel_spmd(nc, in_maps, list(range(BS)), trace=trace)
    LAST_RESULT = res
    out = np.stack([np.asarray(res.results[b]["out_b"]).reshape(N, DK) for b in range(BS)])
    return out.astype(np.float32)



# revision 11
# speedup vs baseline: 1.4641x; 1.0071x over previous
"""MixtureOfSoftMaxACF Trainium2 kernel.

Per-core (data-parallel over BS=8 across 8 cores, batch b per core):
  qt[b] memory reinterpreted as QQ[2, 2048, 64] (contiguous halves), same kt.
  For m in {0,1}:  S_m = QQ[m] @ KK[m].T / sqrt(128);  P_m = softmax(S_m, axis=-1)
  out[b] = (p0 * P_0 + p1 * P_1) @ vt[b]
  p: mixture prior (softmax over batch axis) -> computed on host, passed per-core.

Device pipeline per core:
  - Stage qt/kt as [128, 16, (m,d)] so one PE transpose per key-chunk yields
    both mixtures' d-major columns partition-aligned with the QT/KT layout
    (rows 0-63 = mixture 0 d's, 64-127 = mixture 1); DVE-copy PSUM->SBUF.
  - Scores: S^T [128 keys, 1024 q] = lhsT(K^T chunk [64,128]) @ rhs(Q^T slab),
    bf16 weights/moving (2x PE stream rate vs fp32r), fp32 PSUM accumulate.
  - exp on ScalarE straight from PSUM -> E in SBUF (bf16), scale=1/sqrt(128).
  - AV (V-stationary): outT[128 dv, q] += V_c-stationary matmul, rhs=E, N=512.
  - Denominator: D_rep[128, q] += ones[128,128]-stationary @ E (each row = D).
  - Chunk loop is software-pipelined: scores(c+1) is emitted on PE before
    AV(c)/Drep(c) so the in-order PE queue never idles waiting for exp(c).
  - 1/D via vector.reciprocal_approx_fast (5x faster than DVE reciprocal);
    staging DMAs split 4-way and output stores streamed per 256-q piece.
  - Normalize in the [dv, q] domain (partition-aligned elementwise), combine
    mixtures with prior, PE-transpose back to [q, dv], DVE copy, DMA out.
"""

import math
from contextlib import ExitStack

import numpy as np

import concourse.bass as bass
import concourse.bacc as bacc
import concourse.mybir as mybir
import concourse.tile as tile
from concourse.bass_utils import run_bass_kernel_spmd
from concourse.masks import make_identity

BS = 8
N = 2048          # queries
NK = 2048         # keys
DK = 128
M = 2
D = DK // M       # 64
DV = 128
TEMP = math.sqrt(DK)
NCH = NK // 128   # 16 key chunks
QH = 2            # query halves
QHN = N // QH     # 1024

F32 = mybir.dt.float32
F32R = mybir.dt.float32r
BF16 = mybir.dt.bfloat16

_NC = None
LAST_RESULT = None  # BassKernelResults of last run (test.py reads this)


def _build():
    nc = bacc.Bacc(None)
    qt_d = nc.declare_dram_parameter("qt_b", [N, DK], F32, isOutput=False)
    kt_d = nc.declare_dram_parameter("kt_b", [NK, DK], F32, isOutput=False)
    vt_d = nc.declare_dram_parameter("vt_b", [NK, DK], F32, isOutput=False)
    pr_d = nc.declare_dram_parameter("pr_b", [1, M], F32, isOutput=False)
    out_d = nc.declare_dram_parameter("out_b", [N, DK], F32, isOutput=True)

    with ExitStack() as ctx:
        tc = ctx.enter_context(tile.TileContext(nc))
        const = ctx.enter_context(tc.tile_pool(name="const", bufs=1))
        sbig = ctx.enter_context(tc.tile_pool(name="sbig", bufs=1))
        epool = ctx.enter_context(tc.tile_pool(name="epool", bufs=3))
        npool = ctx.enter_context(tc.tile_pool(name="npool", bufs=2))
        ps_s = ctx.enter_context(tc.tile_pool(name="ps_s", bufs=2, space="PSUM"))
        ps_acc = ctx.enter_context(tc.tile_pool(name="ps_acc", bufs=1, space="PSUM"))
        ps_d = ctx.enter_context(tc.tile_pool(name="ps_d", bufs=1, space="PSUM"))

        # ---- constants ----
        ident_f = const.tile([128, 128], F32)
        make_identity(nc, ident_f)
        ones_f = const.tile([128, 128], F32)
        nc.vector.memset(ones_f, 1.0)
        ones_w = const.tile([128, 128], BF16)
        nc.vector.tensor_copy(ones_w, ones_f)
        pr_sb = const.tile([128, M], F32)
        nc.sync.dma_start(
            out=pr_sb,
            in_=bass.AP(tensor=pr_d, offset=0, ap=[[0, 128], [1, M]]),
        )

        # ---- input staging: [128, 16, (m,d)] so stage[:, c, :] is a [128, 128]
        # block whose transpose has mixture m's d-rows at partitions m*64..m*64+63.
        # stage[p, c, m*64+d] = flat[m*131072 + (c*128+p)*64 + d]
        stages = []
        for src in (qt_d, kt_d):
            t = sbig.tile([128, NCH, DK], F32, tag=f"stage{len(stages)}")
            for m in range(M):
                for h in range(4):
                    hc = NCH // 4
                    nc.sync.dma_start(
                        out=t[:, h * hc:(h + 1) * hc, m * D:(m + 1) * D],
                        in_=bass.AP(
                            tensor=src, offset=m * N * D + h * hc * 128 * D,
                            ap=[[D, 128], [128 * D, hc], [1, D]],
                        ),
                    )
            stages.append(t)

        # V: [128, 16, 128]  (p, c, dv) <- vt[c*128+p, dv]
        v_st = sbig.tile([128, NCH, DV], F32)
        nc.sync.dma_start(
            out=v_st,
            in_=bass.AP(tensor=vt_d, offset=0,
                        ap=[[DK, 128], [128 * DK, NCH], [1, DV]]),
        )
        v_sb = sbig.tile([128, NCH, DV], BF16)
        nc.vector.tensor_copy(v_sb, v_st)

        # ---- phase 1: QT/KT [128, 2048] (rows m*64+d), via PE transpose + DVE copy ----
        qt_t = sbig.tile([128, N], BF16)
        kt_t = sbig.tile([128, NK], BF16)
        for stage, dst in ((stages[0], qt_t), (stages[1], kt_t)):
            for c in range(NCH):
                tp = ps_s.tile([128, 128], F32, tag="s")
                nc.tensor.transpose(tp, stage[:, c, :], ident_f)
                nc.vector.tensor_copy(dst[:, c * 128:(c + 1) * 128], tp)

        # ---- phase 2+3: attention ----
        scale = 1.0 / TEMP
        for qh in range(QH):
            outTn = []
            for m in range(M):
                outT = ps_acc.tile([128, QHN], F32, tag="outT")
                Drep = ps_d.tile([128, QHN], F32, tag="D")
                def emit_scores(c):
                    s = ps_s.tile([128, QHN], F32, tag="s")
                    for hf in range(2):
                        sl = slice(hf * 512, (hf + 1) * 512)
                        nc.tensor.matmul(
                            s[:, sl],
                            lhsT=kt_t[m * D:(m + 1) * D, c * 128:(c + 1) * 128],
                            rhs=qt_t[m * D:(m + 1) * D,
                                     qh * QHN + hf * 512: qh * QHN + (hf + 1) * 512],
                            start=True, stop=True,
                        )
                    return s

                # software pipeline: emit scores(c+1) on PE *before* AV(c)/
                # Drep(c) so the PE queue has work while ACT runs exp(c).
                s_cur = emit_scores(0)
                for c in range(NCH):
                    s_next = emit_scores(c + 1) if c + 1 < NCH else None
                    E = epool.tile([128, QHN], BF16, tag="E")
                    nc.scalar.activation(E, s_cur, mybir.ActivationFunctionType.Exp,
                                         scale=scale)
                    for hf in range(2):
                        sl = slice(hf * 512, (hf + 1) * 512)
                        nc.tensor.matmul(outT[:, sl], lhsT=v_sb[:, c, :], rhs=E[:, sl],
                                         start=(c == 0), stop=(c == NCH - 1))
                        nc.tensor.matmul(Drep[:, sl], lhsT=ones_w, rhs=E[:, sl],
                                         start=(c == 0), stop=(c == NCH - 1))
                    s_cur = s_next
                # normalize this mixture in the [dv, q] domain
                drec = npool.tile([128, QHN], F32, tag="drec")
                nc.vector.reciprocal_approx_fast(drec, Drep)
                otn = npool.tile([128, QHN], F32, tag=f"outTn{m}")
                nc.vector.tensor_mul(otn, outT, drec)
                outTn.append(otn)

            # combine mixtures with prior weights: rT2 = p0*outTn0 + p1*outTn1
            rT = npool.tile([128, QHN], F32, tag="rT")
            nc.vector.tensor_scalar_mul(rT, outTn[0], pr_sb[:, 0:1])
            rT2 = npool.tile([128, QHN], F32, tag="rT2")
            nc.vector.scalar_tensor_tensor(
                out=rT2, in0=outTn[1], scalar=pr_sb[:, 1:2], in1=rT,
                op0=mybir.AluOpType.mult, op1=mybir.AluOpType.add,
            )
            # transpose back to [q, dv], copy to SBUF, store — streamed per
            # 256-q piece so the final store DMA overlaps the transposes.
            res_ps = ps_s.tile([128, QHN], F32, tag="s")
            res_sb = npool.tile([128, QHN], F32, tag="res")
            res_v = res_sb.rearrange("p (t d) -> p t d", d=DV)
            for g in range(QHN // 256):
                for t in (2 * g, 2 * g + 1):
                    nc.tensor.transpose(res_ps[:, t * 128:(t + 1) * 128],
                                        rT2[:, t * 128:(t + 1) * 128], ident_f)
                sl = slice(g * 256, (g + 1) * 256)
                nc.vector.tensor_copy(res_sb[:, sl], res_ps[:, sl])
                nc.sync.dma_start(
                    out=bass.AP(tensor=out_d,
                                offset=(qh * QHN + g * 256) * DK,
                                ap=[[DK, 128], [128 * DK, 2], [1, DV]]),
                    in_=res_v[:, 2 * g:2 * g + 2, :],
                )
    return nc


def _get_nc():
    global _NC
    if _NC is None:
        _NC = _build()
        _NC.finalize()  # Bacc.compile(): event sems, reg alloc, wait legalization
    return _NC


def _prior(qt, kernel):
    bar_qt = qt.astype(np.float32).mean(axis=1)          # (BS, dk)
    logits = kernel.astype(np.float32) @ bar_qt.T        # (m, BS)
    z = logits - logits.max(axis=1, keepdims=True)
    ez = np.exp(z)
    pm = ez / ez.sum(axis=1, keepdims=True)              # softmax over batch axis
    return pm.reshape(-1)


def kernel(qt, kt, vt, kernel):
    global LAST_RESULT
    import os
    nc = _get_nc()
    prior_flat = _prior(qt, kernel)
    in_maps = []
    for b in range(BS):
        pr = np.array([[prior_flat[2 * b], prior_flat[2 * b + 1]]], dtype=np.float32)
        in_maps.append({
            "qt_b": np.ascontiguousarray(qt[b], dtype=np.float32),
            "kt_b": np.ascontiguousarray(kt[b], dtype=np.float32),
            "vt_b": np.ascontiguousarray(vt[b], dtype=np.float32),
            "pr_b": pr,
        })
    trace = bool(int(os.environ.get("KERNEL_TRACE", "0")))
    res = run_bass_kernel_spmd(nc, in_maps, list(range(BS)), trace=trace)
    LAST_RESULT = res
    out = np.stack([np.asarray(res.results[b]["out_b"]).reshape(N, DK) for b in range(BS)])
    return out.astype(np.float32)

